# revision 13
# baseline (speedup 1.0000x reference)
# Trainium2 Bass kernel for nn_DiffNet — v7.
#
# Math (identical reduction to v5): with coef = (conv2_w @ conv1_w)[0],
# bc = conv2_w@conv1_b + conv2_b, scale = RATE/batch_num,
# C* = scale*(coef, bc), each layer reduces to
#   z = vi @ W.T
#   s = sum_i vi,  q = sum_i vi^2
#   alpha = 1 + C2*s,  delta = C0*q + Cb*s
#   out = alpha*relu(z + b) + C1*z + delta
# (the C1*b offset is dropped, as in v5 — O(C1^2) ~ 1e-5 of the output.)
#
# Scheduling model (from trace analysis): measured time = end of the
# NEFF's last teardown instruction minus the preamble's first memset;
# everything after our final output-DMA enqueue is ~10us of fixed
# postamble (253 semaphore resets + barriers). DMA completion semaphores
# fire per 8-partition-row descriptor batch, so ANY column slice of a
# tile waits for its WHOLE dma_start. Therefore:
#  * weights stream as consumption-ordered dma_start quanta alternating
#    across both HWDGE rings (sync+scalar), so matmuls chase the stream
#    at quantum granularity and W1 uses both rings' aggregate bandwidth;
#  * xT and the bias block are tiny single-descriptor (single_packet)
#    DMAs from their own contiguous dram tensors;
#  * each layer's PSUM is split into half-tiles so the relu/epilogue of
#    half 1 runs under the matmuls of half 2 (whole-tile PSUM deps would
#    otherwise serialize reader-after-all-matmuls);
#  * s and q come from ONE memset ones stationary (no constant*ones
#    uploads); alpha/delta use tensor_scalar float immediates;
#  * the per-chunk relu carries the layer bias as a per-partition AP, so
#    there is no bias matmul;
#  * the scalar engine (relu, sum-sq, C0*q) and vector engine (the
#    C1*z+delta chain) run in parallel per half.
#
# Sharding: data-parallel over batch (64 -> 8 rows/core), weights
# replicated, zero collectives. Host transposes the per-core [128,16]
# result back.

import numpy as np

RATE = 0.01
B, IN, H1, H2, OUT = 64, 1024, 512, 512, 256
NCORES = 8
BL = B // NCORES
P128 = 128

NK = [IN // P128, H1 // P128, H2 // P128]    # 8, 4, 4
NCH = [H1 // P128, H2 // P128, OUT // P128]  # 4, 4, 2

# chunk j of layer l (j = c*nk + k) lives on ring j%2 at position j//2.
# Per-ring per-layer dma quanta, in chunks-per-ring units (sum = total/2):
QUANTA = [[4, 4, 8], [8], [4]]
WR_LEN = [NCH[l] * NK[l] * P128 // 2 for l in range(3)]  # 2048, 1024, 512
WR_OFF = [0, 2048, 3072]
WR_TOT = 3584

XT_LEN = NK[0] * BL  # 64
BH_COL = [0, 4, 8]
BH_LEN = 16

N_WARMUP = 12

_NC_CACHE = {}
# per-batch scalar constants (C0, C1, C2, Cb) baked into the compiled
# kernel as immediates; host_prep fills this before get_nc() builds.
_CONSTS = {}


def _build_nc():
    import concourse.bacc as bacc
    import concourse.mybir as mybir
    import concourse.tile as tile
    from concourse.bass import AP

    fp32 = mybir.dt.float32
    fp16 = mybir.dt.float16
    AF = mybir.ActivationFunctionType
    ALU = mybir.AluOpType

    nc = bacc.Bacc("TRN2", target_bir_lowering=False, debug=False)

    xt_t = nc.dram_tensor("xt", [P128, XT_LEN], fp16, kind="ExternalInput")
    wa_t = nc.dram_tensor("wa", [P128, WR_TOT], fp16, kind="ExternalInput")
    wb_t = nc.dram_tensor("wb", [P128, WR_TOT], fp16, kind="ExternalInput")
    bh_t = nc.dram_tensor("bh", [P128, BH_LEN], fp32, kind="ExternalInput")
    out_t = nc.dram_tensor("outT", [P128, 2 * BL], fp32, kind="ExternalOutput")

    with tile.TileContext(nc) as tc:
        with (
            tc.tile_pool(name="wp", bufs=1) as wp,
            tc.tile_pool(name="ap", bufs=1) as ap_,
            tc.tile_pool(name="xp", bufs=1, space="PSUM") as xp,
            tc.tile_pool(name="pp", bufs=3, space="PSUM") as pp,
            tc.tile_pool(name="sp", bufs=2, space="PSUM") as sp,
            tc.tile_pool(name="qp", bufs=2, space="PSUM") as qp,
        ):
            # --- DMAs: enqueue order == transfer order per ring ---
            xt = wp.tile([P128, XT_LEN], fp16, tag="xt")
            nc.sync.dma_start(xt[:], xt_t[:], single_packet=True)
            bhr = ap_.tile([P128, BH_LEN], fp32, tag="bhr")
            nc.scalar.dma_start(bhr[:], bh_t[:], single_packet=True)

            # weight quanta: wq[l][ring] = list of (tile, chunk_pos_base)
            engines = [nc.sync, nc.scalar]
            wq = []
            for l in range(3):
                per_ring = []
                for r, (eng, wt) in enumerate(zip(engines, (wa_t, wb_t))):
                    segs, base = [], 0
                    for ncols in QUANTA[l]:
                        t = wp.tile([P128, ncols * P128], fp16, tag=f"w{l}{r}{base}")
                        lo = WR_OFF[l] + base * P128
                        eng.dma_start(t[:], wt[:, lo : lo + ncols * P128])
                        segs.append((base, ncols, t))
                        base += ncols
                    per_ring.append(segs)
                wq.append(per_ring)

            def wchunk(l, j):
                """Stationary [128,128] chunk j (= c*nk+k) of layer l."""
                r, p = j % 2, j // 2
                for base, ncols, t in wq[l][r]:
                    if base <= p < base + ncols:
                        return t[:, (p - base) * P128 : (p - base + 1) * P128]
                raise AssertionError("bad chunk")

            # --- small on-device constants ---
            ones = wp.tile([P128, P128], fp16, tag="ones")
            nc.gpsimd.memset(ones[:], 1.0)
            junk_a = wp.tile([BL, BL], fp16, tag="junk_a")
            junk_w = wp.tile([BL, BL], fp16, tag="junk_w")
            nc.gpsimd.memset(junk_a[:], 0.0)
            nc.gpsimd.memset(junk_w[:], 0.0)

            # PE warm-up (HAM clock gate) while the DMAs stream
            warm = xp.tile([BL, BL], fp32, tag="warm")
            for _ in range(N_WARMUP):
                nc.tensor.matmul(warm[:], junk_a[:], junk_w[:], start=True, stop=True)

            sq1 = ap_.tile([P128, XT_LEN], fp16, tag="sq1")
            nc.scalar.activation(out=sq1[:], in_=xt[:], func=AF.Square)

            def bcast(t, nch):
                """[128, 8] tile slice -> [128, nch, 8] 0-stride broadcast."""
                return AP(t.tensor, t.offset, [t.ap[0], [0, nch], t.ap[1]])

            C = _CONSTS  # dict: C0, C1, C2, Cb — set by host_prep

            def layer(l, viT, sq):
                nk, nch = NK[l], NCH[l]
                nh = nch // 2  # chunks per half
                hw = nh * BL   # columns per half
                last = l == 2

                bcs = sp.tile([P128, BL], fp32, tag="bcs")
                bcq = qp.tile([P128, BL], fp32, tag="bcq")

                def bcs_mms():
                    for k in range(nk):
                        nc.tensor.matmul(
                            bcs[:], ones[:], viT[:, k * BL : (k + 1) * BL],
                            start=(k == 0), stop=(k == nk - 1),
                        )

                def bcq_mms():
                    for k in range(nk):
                        nc.tensor.matmul(
                            bcq[:], ones[:], sq[:, k * BL : (k + 1) * BL],
                            start=(k == 0), stop=(k == nk - 1),
                        )

                Ph = []
                for _h in range(2):
                    Pht = pp.tile([P128, hw], fp32, tag="P")
                    Ph.append(Pht)

                def pt_mms(h):
                    for ci in range(nh):
                        c = h * nh + ci
                        for k in range(nk):
                            nc.tensor.matmul(
                                Ph[h][:, ci * BL : (ci + 1) * BL],
                                wchunk(l, c * nk + k),
                                viT[:, k * BL : (k + 1) * BL],
                                start=(k == 0), stop=(k == nk - 1),
                            )

                # tensor-engine order: L1 can do bc early (xT is in long
                # before the weights); later layers run Pt first.
                if l == 0:
                    bcs_mms()
                    bcq_mms()
                    pt_mms(0)
                    pt_mms(1)
                else:
                    pt_mms(0)
                    pt_mms(1)
                    bcs_mms()
                    bcq_mms()

                # alpha = 1 + C2*s (vector); delta = C0*q (scalar) + Cb*s
                al = ap_.tile([P128, BL], fp32, tag=f"al{l}")
                nc.vector.tensor_scalar(
                    al[:], bcs[:], C["C2"], 1.0, ALU.mult, ALU.add
                )
                d2 = ap_.tile([P128, BL], fp32, tag=f"d2{l}")
                nc.vector.tensor_scalar(d2[:], bcs[:], C["Cb"], None, ALU.mult)
                d1 = ap_.tile([P128, BL], fp32, tag=f"d1{l}")
                nc.scalar.activation(out=d1[:], in_=bcq[:], func=AF.Copy, scale=C["C0"])
                dl = ap_.tile([P128, BL], fp32, tag=f"dl{l}")
                nc.vector.tensor_tensor(dl[:], d1[:], d2[:], ALU.add)

                o = ap_.tile([P128, nch * BL], fp32 if last else fp16, tag=f"o{l}")
                sqn = (
                    None if last
                    else ap_.tile([P128, nch * BL], fp16, tag=f"sqn{l}")
                )
                for h in range(2):
                    # per-chunk relu with per-partition bias
                    R = ap_.tile([P128, hw], fp32, tag=f"R{l}{h}")
                    for ci in range(nh):
                        col = BH_COL[l] + h * nh + ci
                        nc.scalar.activation(
                            out=R[:, ci * BL : (ci + 1) * BL],
                            in_=Ph[h][:, ci * BL : (ci + 1) * BL],
                            func=AF.Relu,
                            bias=bhr[:, col : col + 1],
                        )
                    t2 = ap_.tile([P128, hw], fp32, tag=f"t2{l}{h}")
                    nc.vector.tensor_scalar(t2[:], Ph[h][:], C["C1"], None, ALU.mult)
                    t3 = ap_.tile([P128, hw], fp32, tag=f"t3{l}{h}")
                    nc.vector.tensor_tensor(t3[:], t2[:], bcast(dl, nh), ALU.add)
                    t4 = ap_.tile([P128, hw], fp32, tag=f"t4{l}{h}")
                    nc.vector.tensor_tensor(t4[:], R[:], bcast(al, nh), ALU.mult)
                    oh = o[:, h * hw : (h + 1) * hw]
                    nc.vector.tensor_tensor(oh, t3[:], t4[:], ALU.add)
                    if last:
                        eng = nc.scalar if h == 0 else nc.sync
                        eng.dma_start(
                            out_t[:, h * hw : (h + 1) * hw], oh, single_packet=True
                        )
                    else:
                        nc.scalar.activation(
                            out=sqn[:, h * hw : (h + 1) * hw], in_=oh, func=AF.Square
                        )
                return o, sqn

            o1, sq2 = layer(0, xt[:], sq1[:])
            o2, sq3 = layer(1, o1[:], sq2[:])
            layer(2, o2[:], sq3[:])

    nc.compile()
    return nc


def get_nc():
    assert _CONSTS, "call host_prep() before get_nc() — constants are baked in"
    key = tuple(sorted(_CONSTS.items()))
    if _NC_CACHE.get("key") != key:
        _NC_CACHE["nc"] = _build_nc()
        _NC_CACHE["key"] = key
    return _NC_CACHE["nc"]


def host_prep(x, fc1_w, fc1_b, fc2_w, fc2_b, fc3_w, fc3_b,
              conv1_w, conv1_b, conv2_w, conv2_b, batch_num):
    f32, f16, f64 = np.float32, np.float16, np.float64
    x = np.asarray(x, f32)
    ws = [np.asarray(fc1_w, f32), np.asarray(fc2_w, f32), np.asarray(fc3_w, f32)]
    bs = [np.asarray(fc1_b, f32), np.asarray(fc2_b, f32), np.asarray(fc3_b, f32)]

    bn = float(np.asarray(batch_num).item())
    scale = RATE / bn
    coef = (np.asarray(conv2_w, f64) @ np.asarray(conv1_w, f64))[0]
    bcv = float(
        (np.asarray(conv2_w, f64) @ np.asarray(conv1_b, f64))[0]
        + np.asarray(conv2_b, f64)[0]
    )
    C0, C1, C2 = (scale * coef).astype(f64)
    Cb = scale * bcv
    _CONSTS.clear()
    _CONSTS.update(
        {"C0": float(C0), "C1": float(C1), "C2": float(C2), "Cb": float(Cb)}
    )

    bh = np.zeros((P128, BH_LEN), f32)
    for l in range(3):
        for c in range(NCH[l]):
            bh[:, BH_COL[l] + c] = bs[l][c * P128 : (c + 1) * P128]

    wa = np.zeros((P128, WR_TOT), f16)
    wb = np.zeros((P128, WR_TOT), f16)
    for l in range(3):
        Wt = ws[l].T.astype(f16)  # [in, out]
        nk, nch = NK[l], NCH[l]
        for j in range(nch * nk):
            c, k = j // nk, j % nk
            chunk = Wt[k * P128 : (k + 1) * P128, c * P128 : (c + 1) * P128]
            p = j // 2
            dst = wa if j % 2 == 0 else wb
            dst[:, WR_OFF[l] + p * P128 : WR_OFF[l] + (p + 1) * P128] = chunk

    in_maps = []
    for kcore in range(NCORES):
        xk = x[kcore * BL : (kcore + 1) * BL]
        xt = (
            xk.T.reshape(NK[0], P128, BL).transpose(1, 0, 2).reshape(P128, XT_LEN)
        ).astype(f16)
        in_maps.append(
            {"xt": np.ascontiguousarray(xt), "wa": wa, "wb": wb, "bh": bh}
        )
    return in_maps


def _unshard(outT):
    """[128, 16] -> [8, 256]: out[b, c*128+p] = outT[p, c*8+b]."""
    return np.ascontiguousarray(
        outT.reshape(P128, 2, BL).transpose(2, 1, 0).reshape(BL, OUT), dtype=np.float32
    )


def kernel(**inputs):
    from concourse.bass_utils import run_bass_kernel_spmd

    in_maps = host_prep(**inputs)
    nc = get_nc()
    res = run_bass_kernel_spmd(nc, in_maps, core_ids=list(range(NCORES)))
    out = np.concatenate(
        [_unshard(res.results[k]["outT"]) for k in range(NCORES)], axis=0
    )
    return np.ascontiguousarray(out, dtype=np.float32)


# revision 14
# speedup vs baseline: 1.0095x; 1.0095x over previous
# Trainium2 Bass kernel for nn_DiffNet — v8.
#
# Math (identical reduction to v5): with coef = (conv2_w @ conv1_w)[0],
# bc = conv2_w@conv1_b + conv2_b, scale = RATE/batch_num,
# C* = scale*(coef, bc), each layer reduces to
#   z = vi @ W.T
#   s = sum_i vi,  q = sum_i vi^2
#   alpha = 1 + C2*s,  delta = C0*q + Cb*s
#   out = alpha*relu(z + b) + C1*z + delta
# (the C1*b offset is dropped, as in v5 — O(C1^2) ~ 1e-5 of the output.)
#
# Scheduling model (from trace analysis):
#  * measured time = end of the NEFF's last teardown instruction minus
#    the preamble's first memset; everything after our final output-DMA
#    enqueue is ~10us of fixed postamble. So the objective is simply to
#    enqueue the output DMA as early as possible.
#  * DMA completion fires per dma_start (the 16 sem increments batch
#    8 partition-rows each, so any column slice waits the whole DMA),
#    and ring bandwidth (~220GB/s each) needs the 128-descriptor form
#    (16 DMA engines in parallel; single-descriptor DMAs run ~25GB/s).
#    => one big dma_start per LAYER-HALF per ring, halves split across
#    the two rings so each half completes in half the layer time and
#    the two halves of a layer land simultaneously.
#  * DMA enqueue costs ~565-667ns of the issuing engine's queue, so the
#    scalar engine's compute duties (sq1, C0*q) moved to vector/gpsimd.
#  * each layer's PSUM is split into half-tiles so the relu/epilogue of
#    half 1 runs under the matmuls of half 2.
#  * s and q come from ONE memset ones stationary; alpha/delta use
#    tensor_scalar float immediates; the per-chunk relu carries the
#    layer bias as a per-partition AP (no bias matmul).
#  * epilogue engines: scalar = relu + next-layer sum-squares,
#    vector = alpha/delta + C1*z+delta chain, gpsimd = R*alpha and the
#    final add (SBUF-only operands), so the three chains overlap.
#
# Sharding: data-parallel over batch (64 -> 8 rows/core), weights
# replicated, zero collectives. Host transposes the per-core [128,16]
# result back.

import numpy as np

RATE = 0.01
B, IN, H1, H2, OUT = 64, 1024, 512, 512, 256
NCORES = 8
BL = B // NCORES
P128 = 128

NK = [IN // P128, H1 // P128, H2 // P128]    # 8, 4, 4
NCH = [H1 // P128, H2 // P128, OUT // P128]  # 4, 4, 2

# ring r carries half r of every layer: chunks j = c*nk + k with
# c in [r*nch/2, (r+1)*nch/2); stored at ring-local position
# (c - r*nch/2)*nk + k. One dma_start per layer per ring.
WR_LEN = [NCH[l] * NK[l] * P128 // 2 for l in range(3)]  # 2048, 1024, 512
WR_OFF = [0, 2048, 3072]
WR_TOT = 3584

XT_LEN = NK[0] * BL  # 64
BH_COL = [0, 4, 8]
BH_LEN = 16

N_WARMUP = 12

_NC_CACHE = {}
# per-batch scalar constants (C0, C1, C2, Cb) baked into the compiled
# kernel as immediates; host_prep fills this before get_nc() builds.
_CONSTS = {}


def _build_nc():
    import concourse.bacc as bacc
    import concourse.mybir as mybir
    import concourse.tile as tile
    from concourse.bass import AP

    fp32 = mybir.dt.float32
    fp16 = mybir.dt.float16
    AF = mybir.ActivationFunctionType
    ALU = mybir.AluOpType

    nc = bacc.Bacc("TRN2", target_bir_lowering=False, debug=False)

    xt_t = nc.dram_tensor("xt", [P128, XT_LEN], fp16, kind="ExternalInput")
    wa_t = nc.dram_tensor("wa", [P128, WR_TOT], fp16, kind="ExternalInput")
    wb_t = nc.dram_tensor("wb", [P128, WR_TOT], fp16, kind="ExternalInput")
    bh_t = nc.dram_tensor("bh", [P128, BH_LEN], fp32, kind="ExternalInput")
    out_t = nc.dram_tensor("outT", [P128, 2 * BL], fp32, kind="ExternalOutput")

    with tile.TileContext(nc) as tc:
        with (
            tc.tile_pool(name="wp", bufs=1) as wp,
            tc.tile_pool(name="ap", bufs=1) as ap_,
            tc.tile_pool(name="xp", bufs=1, space="PSUM") as xp,
            tc.tile_pool(name="pp", bufs=3, space="PSUM") as pp,
            tc.tile_pool(name="sp", bufs=2, space="PSUM") as sp,
            tc.tile_pool(name="qp", bufs=2, space="PSUM") as qp,
        ):
            # --- DMAs: enqueue order == transfer order per ring ---
            xt = wp.tile([P128, XT_LEN], fp16, tag="xt")
            nc.sync.dma_start(xt[:], xt_t[:], single_packet=True)
            bhr = ap_.tile([P128, BH_LEN], fp32, tag="bhr")
            nc.scalar.dma_start(bhr[:], bh_t[:], single_packet=True)

            wseg = [[], []]  # wseg[r][l] -> tile of half r of layer l
            for l in range(3):
                for r, (eng, wt) in enumerate(
                    zip((nc.sync, nc.scalar), (wa_t, wb_t))
                ):
                    t = wp.tile([P128, WR_LEN[l]], fp16, tag=f"w{l}{r}")
                    eng.dma_start(t[:], wt[:, WR_OFF[l] : WR_OFF[l] + WR_LEN[l]])
                    wseg[r].append(t)

            def wchunk(l, c, k):
                """Stationary [128,128] chunk (c,k) of layer l."""
                nh = NCH[l] // 2
                r, cl = (0, c) if c < nh else (1, c - nh)
                p = cl * NK[l] + k
                return wseg[r][l][:, p * P128 : (p + 1) * P128]

            # --- small on-device constants ---
            ones = wp.tile([P128, P128], fp16, tag="ones")
            nc.gpsimd.memset(ones[:], 1.0)
            junk_a = wp.tile([BL, BL], fp16, tag="junk_a")
            junk_w = wp.tile([BL, BL], fp16, tag="junk_w")
            nc.gpsimd.memset(junk_a[:], 0.0)
            nc.gpsimd.memset(junk_w[:], 0.0)

            # PE warm-up (HAM clock gate) while the DMAs stream
            warm = xp.tile([BL, BL], fp32, tag="warm")
            for _ in range(N_WARMUP):
                nc.tensor.matmul(warm[:], junk_a[:], junk_w[:], start=True, stop=True)

            # x^2 on vector (scalar is busy enqueueing weight DMAs)
            sq1 = ap_.tile([P128, XT_LEN], fp16, tag="sq1")
            nc.vector.tensor_tensor(sq1[:], xt[:], xt[:], ALU.mult)

            def bcast(t, nch):
                """[128, 8] tile slice -> [128, nch, 8] 0-stride broadcast."""
                return AP(t.tensor, t.offset, [t.ap[0], [0, nch], t.ap[1]])

            C = _CONSTS  # dict: C0, C1, C2, Cb — set by host_prep

            def layer(l, viT, sq):
                nk, nch = NK[l], NCH[l]
                nh = nch // 2  # chunks per half
                hw = nh * BL   # columns per half
                last = l == 2

                bcs = sp.tile([P128, BL], fp32, tag="bcs")
                bcq = qp.tile([P128, BL], fp32, tag="bcq")

                def bcs_mms():
                    for k in range(nk):
                        nc.tensor.matmul(
                            bcs[:], ones[:], viT[:, k * BL : (k + 1) * BL],
                            start=(k == 0), stop=(k == nk - 1),
                        )

                def bcq_mms():
                    for k in range(nk):
                        nc.tensor.matmul(
                            bcq[:], ones[:], sq[:, k * BL : (k + 1) * BL],
                            start=(k == 0), stop=(k == nk - 1),
                        )

                Ph = []
                for _h in range(2):
                    Pht = pp.tile([P128, hw], fp32, tag="P")
                    Ph.append(Pht)

                def pt_mms(h):
                    for ci in range(nh):
                        c = h * nh + ci
                        for k in range(nk):
                            nc.tensor.matmul(
                                Ph[h][:, ci * BL : (ci + 1) * BL],
                                wchunk(l, c, k),
                                viT[:, k * BL : (k + 1) * BL],
                                start=(k == 0), stop=(k == nk - 1),
                            )

                # tensor-engine order: L1 can do bc early (xT is in long
                # before the weights); later layers run Pt first.
                if l == 0:
                    bcs_mms()
                    bcq_mms()
                    pt_mms(0)
                    pt_mms(1)
                else:
                    pt_mms(0)
                    pt_mms(1)
                    bcs_mms()
                    bcq_mms()

                # alpha = 1 + C2*s; delta = C0*q + Cb*s   (vector)
                al = ap_.tile([P128, BL], fp32, tag=f"al{l}")
                nc.vector.tensor_scalar(
                    al[:], bcs[:], C["C2"], 1.0, ALU.mult, ALU.add
                )
                d1 = ap_.tile([P128, BL], fp32, tag=f"d1{l}")
                nc.vector.tensor_scalar(d1[:], bcq[:], C["C0"], None, ALU.mult)
                d2 = ap_.tile([P128, BL], fp32, tag=f"d2{l}")
                nc.vector.tensor_scalar(d2[:], bcs[:], C["Cb"], None, ALU.mult)
                dl = ap_.tile([P128, BL], fp32, tag=f"dl{l}")
                nc.vector.tensor_tensor(dl[:], d1[:], d2[:], ALU.add)

                o = ap_.tile([P128, nch * BL], fp32 if last else fp16, tag=f"o{l}")
                sqn = (
                    None if last
                    else ap_.tile([P128, nch * BL], fp16, tag=f"sqn{l}")
                )
                for h in range(2):
                    # per-chunk relu with per-partition bias (scalar)
                    R = ap_.tile([P128, hw], fp32, tag=f"R{l}{h}")
                    for ci in range(nh):
                        col = BH_COL[l] + h * nh + ci
                        nc.scalar.activation(
                            out=R[:, ci * BL : (ci + 1) * BL],
                            in_=Ph[h][:, ci * BL : (ci + 1) * BL],
                            func=AF.Relu,
                            bias=bhr[:, col : col + 1],
                        )
                    t2 = ap_.tile([P128, hw], fp32, tag=f"t2{l}{h}")
                    nc.vector.tensor_scalar(t2[:], Ph[h][:], C["C1"], None, ALU.mult)
                    t3 = ap_.tile([P128, hw], fp32, tag=f"t3{l}{h}")
                    nc.vector.tensor_tensor(t3[:], t2[:], bcast(dl, nh), ALU.add)
                    # gpsimd path reads only SBUF: R*alpha, then +t3
                    t4 = ap_.tile([P128, hw], fp32, tag=f"t4{l}{h}")
                    nc.gpsimd.tensor_tensor(t4[:], R[:], bcast(al, nh), ALU.mult)
                    oh = o[:, h * hw : (h + 1) * hw]
                    nc.gpsimd.tensor_tensor(oh, t3[:], t4[:], ALU.add)
                    if last:
                        eng = nc.scalar if h == 0 else nc.sync
                        eng.dma_start(
                            out_t[:, h * hw : (h + 1) * hw], oh, single_packet=True
                        )
                    else:
                        nc.scalar.activation(
                            out=sqn[:, h * hw : (h + 1) * hw], in_=oh, func=AF.Square
                        )
                return o, sqn

            o1, sq2 = layer(0, xt[:], sq1[:])
            o2, sq3 = layer(1, o1[:], sq2[:])
            layer(2, o2[:], sq3[:])

    nc.compile()
    return nc


def get_nc():
    assert _CONSTS, "call host_prep() before get_nc() — constants are baked in"
    key = tuple(sorted(_CONSTS.items()))
    if _NC_CACHE.get("key") != key:
        _NC_CACHE["nc"] = _build_nc()
        _NC_CACHE["key"] = key
    return _NC_CACHE["nc"]


def host_prep(x, fc1_w, fc1_b, fc2_w, fc2_b, fc3_w, fc3_b,
              conv1_w, conv1_b, conv2_w, conv2_b, batch_num):
    f32, f16, f64 = np.float32, np.float16, np.float64
    x = np.asarray(x, f32)
    ws = [np.asarray(fc1_w, f32), np.asarray(fc2_w, f32), np.asarray(fc3_w, f32)]
    bs = [np.asarray(fc1_b, f32), np.asarray(fc2_b, f32), np.asarray(fc3_b, f32)]

    bn = float(np.asarray(batch_num).item())
    scale = RATE / bn
    coef = (np.asarray(conv2_w, f64) @ np.asarray(conv1_w, f64))[0]
    bcv = float(
        (np.asarray(conv2_w, f64) @ np.asarray(conv1_b, f64))[0]
        + np.asarray(conv2_b, f64)[0]
    )
    C0, C1, C2 = (scale * coef).astype(f64)
    Cb = scale * bcv
    _CONSTS.clear()
    _CONSTS.update(
        {"C0": float(C0), "C1": float(C1), "C2": float(C2), "Cb": float(Cb)}
    )

    bh = np.zeros((P128, BH_LEN), f32)
    for l in range(3):
        for c in range(NCH[l]):
            bh[:, BH_COL[l] + c] = bs[l][c * P128 : (c + 1) * P128]

    wa = np.zeros((P128, WR_TOT), f16)
    wb = np.zeros((P128, WR_TOT), f16)
    for l in range(3):
        Wt = ws[l].T.astype(f16)  # [in, out]
        nk, nh = NK[l], NCH[l] // 2
        for c in range(NCH[l]):
            for k in range(nk):
                chunk = Wt[k * P128 : (k + 1) * P128, c * P128 : (c + 1) * P128]
                r, cl = (0, c) if c < nh else (1, c - nh)
                p = cl * nk + k
                dst = wa if r == 0 else wb
                dst[:, WR_OFF[l] + p * P128 : WR_OFF[l] + (p + 1) * P128] = chunk

    in_maps = []
    for kcore in range(NCORES):
        xk = x[kcore * BL : (kcore + 1) * BL]
        xt = (
            xk.T.reshape(NK[0], P128, BL).transpose(1, 0, 2).reshape(P128, XT_LEN)
        ).astype(f16)
        in_maps.append(
            {"xt": np.ascontiguousarray(xt), "wa": wa, "wb": wb, "bh": bh}
        )
    return in_maps


def _unshard(outT):
    """[128, 16] -> [8, 256]: out[b, c*128+p] = outT[p, c*8+b]."""
    return np.ascontiguousarray(
        outT.reshape(P128, 2, BL).transpose(2, 1, 0).reshape(BL, OUT), dtype=np.float32
    )


def kernel(**inputs):
    from concourse.bass_utils import run_bass_kernel_spmd

    in_maps = host_prep(**inputs)
    nc = get_nc()
    res = run_bass_kernel_spmd(nc, in_maps, core_ids=list(range(NCORES)))
    out = np.concatenate(
        [_unshard(res.results[k]["outT"]) for k in range(NCORES)], axis=0
    )
    return np.ascontiguousarray(out, dtype=np.float32)
